# revision 23
# baseline (speedup 1.0000x reference)
"""Trainium2 Bass kernel for nn_LowpassDetector (4th-order Butterworth IIR
lowpass over [T=65536, C=512], zero initial conditions).

Approach: the filter's slowest pole has |p| = 0.7577, so the impulse
response decays below the needed tolerance after ~32 taps; the sequential
IIR is therefore numerically a short causal FIR.  A 128-sample time block
satisfies

    y_blk[n] = A @ x_blk[n] + B32 @ tail(x_blk[n-1])

where A[i,j] = h[i-j] (banded lower-triangular, 128 taps kept — free) and
B32 covers the 32 straddling taps from the last 32 rows of the previous
block.  A maps to one full TensorEngine matmul per block; the four B32
matmuls of four consecutive blocks are K=32 strips placed at distinct
row-groups via tile_position, so they execute concurrently in one PE pass
(the host stages the four tails at the four partition offsets).

Quantization (rel-err budget 2e-2; this lands at ~5e-3): device IO is
fp8-e3m4 both ways to minimize HBM traffic (memory-bound kernel).  Host
sends v = e3m4(16*(x - 0.5)); weights are plain fp16 conv matrices, so
PSUM holds 16*(y - 0.5*S) in [-10, 10], stored as e3m4 directly.  Host
reconstructs y = y8/16 + 0.5*S[n], where S[n] = cumsum(h)[min(n, 255)] is
the exact DC step response including the zero-state startup ramp (the
mean-subtraction halves the fp8 quantization noise).

Sharding: time axis across the 8 cores (8192 steps each) with a
128-sample halo block from the previous shard (zeros for core 0);
channels (512) ride the matmul free dimension.  Host prepends the halo,
gathers per-core outputs.
"""

from contextlib import ExitStack

import ml_dtypes
import numpy as np

import concourse.mybir as mybir
import concourse.tile as tile
from concourse import bacc
from concourse._compat import get_trn_type
from concourse.bass_utils import run_bass_kernel_spmd

T, C = 65536, 512
NCORES = 8
TL = T // NCORES            # 8192 timesteps per core
B = 128                     # time block (partition dim / conv matrix size)
NBLK = TL // B              # 64 output blocks per core
SUP = 16                    # blocks per superblock DMA
NSUP = NBLK // SUP          # 4 superblocks per core
IN_ROWS = TL + B            # 8320 input rows per core (halo + shard)
GRP = 2                     # blocks per PSUM group tile (2 banks)
UNIT = 4                    # blocks per packed-tail matmul unit
NUNIT = SUP // UNIT         # 4 units per superblock
XW = SUP * C + UNIT * C     # in-superblock free width: 16 blocks + 4 tail tiles

ORDER = 4
CUTOFF = 20e9
SAMPLERATE = 160e9
RESPONSIVITY = 1.0
F32 = mybir.dt.float32
F16 = mybir.dt.float16
F8 = mybir.dt.float8e3
E3M4 = ml_dtypes.float8_e3m4

XSCALE = 16.0               # input quant scale: v = XSCALE * (x - 0.5)


def _butter_lowpass(order, wn):
    """Digital Butterworth lowpass (b, a); same math as the model."""
    fs = 2.0
    warped = 2.0 * fs * np.tan(np.pi * wn / fs)
    m = np.arange(-order + 1, order, 2)
    p = -np.exp(1j * np.pi * m / (2.0 * order))
    p = warped * p
    k = warped**order
    fs2 = 2.0 * fs
    pz = (fs2 + p) / (fs2 - p)
    zz = -np.ones(order)
    kz = k * np.real(1.0 / np.prod(fs2 - p))
    b = np.real(kz * np.poly(zz))
    a = np.real(np.poly(pz))
    return b, a


def _impulse_response():
    b, a = _butter_lowpass(ORDER, 2.0 * CUTOFF / SAMPLERATE)
    # impulse response in float64 via the DFII-T recurrence
    K = 2 * B
    h = np.zeros(K)
    z = np.zeros(ORDER)
    for n in range(K):
        xn = 1.0 if n == 0 else 0.0
        y = b[0] * xn + z[0]
        z = np.concatenate([z[1:], [0.0]]) + b[1:] * xn - a[1:] * y
        h[n] = y
    return h * RESPONSIVITY


def _conv_mats():
    """Block-convolution matrices (float64, untransposed)."""
    h = _impulse_response()
    K = 2 * B
    i = np.arange(B)[:, None]
    j = np.arange(B)[None, :]
    A = np.where(i >= j, h[np.clip(i - j, 0, K - 1)], 0.0)
    Bm = h[i + B - j]  # i + B - j in [1, 2B-1]
    return A, Bm


def build_program():
    nc = bacc.Bacc(get_trn_type() or "TRN2", target_bir_lowering=False, debug=False)

    # x_sb[s, p, b*C + c] = shard[(s*SUP + b)*B + p - B, c] for b < SUP
    # (halo included); the last UNIT*C columns hold the packed tails:
    # x_sb[s, 32*r + p32, (SUP+m)*C + c] = tail row p32 of input block
    # s*SUP + 4*m + r.  Host pre-arranges everything.
    x_sb = nc.dram_tensor("x", [NSUP, B, XW], F8, kind="ExternalInput").ap()
    x_tail = nc.dram_tensor("xt", [B, C], F8, kind="ExternalInput").ap()
    # w[:, 0:B] = A.T (fp16); w[:, B:2B] = tails strip Bm.T[96:128] tiled 4x.
    w_in = nc.dram_tensor("w", [B, 2 * B], F16, kind="ExternalInput").ap()
    # y_raw[s, p, b*C + c] = y8[s*SUP*B + b*B + p, c] — host un-permutes.
    y_out = nc.dram_tensor("y", [NSUP, B, SUP * C], F8, kind="ExternalOutput").ap()

    with ExitStack() as ctx:
        tc = ctx.enter_context(tile.TileContext(nc))
        cpool = ctx.enter_context(tc.tile_pool(name="consts", bufs=1))
        inpool = ctx.enter_context(tc.tile_pool(name="insb", bufs=3))
        outpool = ctx.enter_context(tc.tile_pool(name="outsb", bufs=3))
        pspool = ctx.enter_context(tc.tile_pool(name="ps", bufs=4, space="PSUM"))

        # Vector finishes its preamble early; let it zero the warmup tile
        # so the HAM-warmup matmuls can start immediately.
        wz = cpool.tile([B, 64], F16, tag="warmz", name="warmz")
        nc.vector.memset(wz[:], 0.0)

        # Startup-critical loads on the sync HWDGE queue, in consumption
        # order and small chunks (each dma pays ~2us completion latency).
        in_tiles = {}
        t0 = inpool.tile([B, XW], F8, tag="insb")
        in_tiles[0] = t0
        w_all = cpool.tile([B, 2 * B], F16, tag="w_all", name="w_all")
        nc.sync.dma_start(t0[:, SUP * C : XW], x_sb[0, :, SUP * C : XW])
        nc.sync.dma_start(t0[:, 0 : 2 * C], x_sb[0, :, 0 : 2 * C])
        nc.sync.dma_start(w_all[:], w_in[:])
        nc.sync.dma_start(t0[:, 2 * C : 6 * C], x_sb[0, :, 2 * C : 6 * C])
        nc.sync.dma_start(t0[:, 6 * C : 10 * C], x_sb[0, :, 6 * C : 10 * C])
        nc.sync.dma_start(t0[:, 10 * C : 13 * C], x_sb[0, :, 10 * C : 13 * C])
        nc.sync.dma_start(t0[:, 13 * C : SUP * C], x_sb[0, :, 13 * C : SUP * C])
        w_a = w_all[:, 0:B]
        w_t = w_all[:, B : 2 * B]

        # PE HAM warmup: harmless matmuls on a zeroed tile bridge the
        # ~3.4 us DMA head so the real matmuls start at the warm clock.
        wps = pspool.tile([B, GRP * C], F32, tag="ps")
        for _ in range(84):
            nc.tensor.matmul(
                wps[0:64, 0:64], wz[:, 0:64], wz[:, 0:64], start=True, stop=True
            )

        def load_in_sb(s):
            # All inputs stay on the sync queue (q1), FIFO'd behind the
            # startup chunks.  Each superblock loads in consumption order
            # (packed tails first — the unit's B-matmuls need them — then
            # the first blocks, then the bulk) so the first matmuls of a
            # superblock never wait on the whole 1.3 MB transfer.
            t = inpool.tile([B, XW], F8, tag="insb")
            if s < NSUP:
                nc.sync.dma_start(t[:, SUP * C : XW], x_sb[s, :, SUP * C : XW])
                nc.sync.dma_start(t[:, 0 : 4 * C], x_sb[s, :, 0 : 4 * C])
                nc.sync.dma_start(
                    t[:, 4 * C : SUP * C], x_sb[s, :, 4 * C : SUP * C]
                )
            else:  # tail halo block (input block index NBLK)
                nc.sync.dma_start(t[:, 0:C], x_tail[:])
            in_tiles[s] = t

        def rhs(sb, q):
            return in_tiles[sb][:, q * C : (q + 1) * C]

        load_in_sb(1)
        for s in range(NSUP):
            if s + 2 <= NSUP:
                load_in_sb(s + 2)  # two superblocks of prefetch depth
            out_t = outpool.tile([B, SUP * C], F8, tag="outsb")
            for m in range(NUNIT):
                # Two 2-bank PSUM tiles hold this unit's 4 blocks.  The 4
                # straddle-tap matmuls are K=32 strips at 4 distinct
                # row-groups — they run concurrently in one PE pass.
                psA = pspool.tile([B, GRP * C], F32, tag="ps")
                psB = pspool.tile([B, GRP * C], F32, tag="ps")
                for r in range(UNIT):
                    q = m * UNIT + r
                    pst = psA if r < GRP else psB
                    nc.tensor.matmul(
                        pst[:, (q % GRP) * C : (q % GRP + 1) * C],
                        w_t[32 * r : 32 * r + 32, :],
                        in_tiles[s][32 * r : 32 * r + 32,
                                    (SUP + m) * C : (SUP + m + 1) * C],
                        start=True,
                        stop=False,
                        tile_position=(32 * r, 0),
                    )
                for r in range(UNIT):
                    q = m * UNIT + r
                    j = s * SUP + q  # output block; cur input block j+1
                    cur_sb, cur_q = (j + 1) // SUP, (j + 1) % SUP
                    pst = psA if r < GRP else psB
                    nc.tensor.matmul(
                        pst[:, (q % GRP) * C : (q % GRP + 1) * C],
                        w_a,
                        rhs(cur_sb, cur_q),
                        start=False,
                        stop=True,
                    )
                lo = m * UNIT * C
                nc.vector.tensor_copy(out_t[:, lo : lo + GRP * C], psA[:])
                nc.scalar.activation(
                    out_t[:, lo + GRP * C : lo + UNIT * C],
                    psB[:],
                    mybir.ActivationFunctionType.Copy,
                )
            if s == NSUP - 1:
                # Shrinking output pieces for the final superblock: earlier
                # pieces ship while the last blocks compute, and the last
                # (smallest) piece minimizes the end-of-kernel DMA drain.
                # Alternate sequencers so the issue slots don't serialize,
                # ending with a single-block piece to minimize the final
                # transfer+receipt chain.
                pieces = (
                    (0, 8, nc.sync),
                    (8, 13, nc.scalar),
                    (13, 15, nc.sync),
                    (15, 16, nc.scalar),
                )
                for lo_b, hi_b, e in pieces:
                    e.dma_start(
                        y_out[s, :, lo_b * C : hi_b * C],
                        out_t[:, lo_b * C : hi_b * C],
                    )
            else:
                nc.sync.dma_start(y_out[s], out_t[:])

    nc.compile()
    return nc


_prog = None


def _get_prog():
    global _prog
    if _prog is None:
        _prog = build_program()
    return _prog


def make_in_maps(signal):
    x = np.asarray(signal, dtype=np.float32)
    assert x.shape == (T, C), x.shape
    # mean-subtracted, scaled fp8-e3m4 input (range +-8, e3m4 max 15.5)
    v8 = (XSCALE * (x - 0.5)).astype(E3M4)
    A, Bm = _conv_mats()
    w_tails = np.tile(Bm.T[96:128, :], (4, 1))  # [128, 128]
    w_all = np.ascontiguousarray(
        np.hstack([A.T.astype(np.float16), w_tails.astype(np.float16)])
    )
    in_maps = []
    for c in range(NCORES):
        if c == 0:
            halo = np.zeros((B, C), E3M4)
        else:
            halo = v8[c * TL - B : c * TL]
        xc = np.concatenate([halo, v8[c * TL : (c + 1) * TL]], 0)  # [IN_ROWS, C]
        # main blocks in SBUF tile layout:
        # x_sb[s, p, b*C+c] = xc[(s*SUP + b)*B + p, c]
        x_main = (
            xc[: NSUP * SUP * B]
            .reshape(NSUP, SUP, B, C)
            .transpose(0, 2, 1, 3)
            .reshape(NSUP, B, SUP * C)
        )
        # packed tails: tails[s, 32r+p32, m*C+c] = xc[(s*16+4m+r)*B + 96 + p32, c]
        R = xc[: NBLK * B].reshape(NBLK, B, C)[:, 96:128, :]  # [64, 32, C]
        x_tails = (
            R.reshape(NSUP, NUNIT, UNIT, 32, C)
            .transpose(0, 2, 3, 1, 4)
            .reshape(NSUP, B, UNIT * C)
        )
        x_sbm = np.ascontiguousarray(np.concatenate([x_main, x_tails], axis=2))
        x_tl = np.ascontiguousarray(xc[NBLK * B :])
        in_maps.append({"x": x_sbm, "xt": x_tl, "w": w_all})
    return in_maps


def _dc_offset():
    """off[n] = 0.5 * cumsum(h)[min(n, 255)] — the exact DC term removed by
    the mean-subtraction, including the zero-state startup ramp."""
    h = _impulse_response()
    S = np.cumsum(h)
    idx = np.minimum(np.arange(T), 2 * B - 1)
    return (0.5 * S[idx]).astype(np.float32)


def unpack_y(y_raw):
    """y_raw [NSUP, B, SUP*C] -> [TL, C] (inverse of the tile layout)."""
    return np.ascontiguousarray(
        y_raw.reshape(NSUP, B, SUP, C).transpose(0, 2, 1, 3).reshape(TL, C)
    )


def run(signal, trace=False):
    """Run on the 8 NeuronCores; returns (y, BassKernelResults)."""
    nc = _get_prog()
    in_maps = make_in_maps(signal)
    last_err = None
    for _attempt in range(3):
        try:
            res = run_bass_kernel_spmd(
                nc, in_maps, core_ids=list(range(NCORES)), trace=trace
            )
            break
        except Exception as e:  # transient NRT device errors; retry
            last_err = e
    else:
        raise last_err
    y8 = np.concatenate(
        [unpack_y(np.asarray(res.results[c]["y"])) for c in range(NCORES)], 0
    )
    y = y8.astype(np.float32) * (1.0 / XSCALE) + _dc_offset()[:, None]
    return y, res


def kernel(signal=None, **unused):
    if signal is None:
        signal = unused.pop("signal")
    y, _ = run(signal)
    return y
